# revision 73
# baseline (speedup 1.0000x reference)
"""MinGRU Trainium2 kernel (v4).

Reference computation (per batch b):
    c = depthwise_conv1d(x, conv_w, taps=5, pad=2)        # [D, L]
    h = h_w @ c                                           # [O, L]
    g = concat([-1000, +1000], g_w @ c)                   # [O, L]
    a = sigmoid(-g); v = (1 - a) * h
    out[l] = a[l] * out[l-1] + v[l]     (linear scan along L)

Strategy: pure data-parallel over B (8 batches -> 8 NeuronCores).
Per core, stream in l-chunks of 512:
  - conv: diagonal matmuls on TensorE, packed as 64x64 array tiles: a
    d-tile PAIR runs 4 concurrent tile-position matmuls per tap
    ((0,0),(64,64) for the even d-tile; (0,64),(64,0) for the odd one,
    whose x/weights are partition-rolled by 64 host-side).  Measured
    1.68x over full-width diagonal matmuls.
  - c PSUM->SBUF copies on ScalarE; h/g 1x1-conv matmuls bf16.
  - a = sigmoid(-(g + bias)) on ScalarE (bias carries +/-1000 polarized
    rows 0/1, built on-chip from a partition iota); z = 1 - a on GpSimd;
    v = z*h on VectorE; scan via tensor_tensor_scan (DVE).
  - rows 0/1 come out of the scan naturally (a saturates to exactly 1/0).
  - x is relaid out host-side to [128, chunk, dt, 516] WITH the +/-2
    halos baked in (edges zero-padded), odd d-tiles rolled by 64
    partitions, so each chunk loads with ONE contiguous DMA.
  - out is bf16 (host converts to f32; ~0.2% extra rounding, well under
    the 2e-2 gate), written as [128, (ot, chunk, col)] and permuted
    host-side; chunk 7 runs its sigmoid/z/v/scan chain on 256-col halves
    with per-o-tile stores to shorten the tail.
"""

import numpy as np
import ml_dtypes

import concourse.bass as bass
import concourse.mybir as mybir
from concourse import bacc
from concourse.tile import TileContext
from concourse.bass_utils import run_bass_kernel_spmd

F32 = mybir.dt.float32
BF16 = mybir.dt.bfloat16
F8E4 = mybir.dt.float8e4
I32 = mybir.dt.int32
AF = mybir.ActivationFunctionType
OP = mybir.AluOpType
PM = mybir.MatmulPerfMode

B, D, O, L = 8, 512, 512, 4096
P = 128
CH = 512                 # l-chunk width (one PSUM bank)
NCH = L // CH            # 8
NDT = D // P             # 4 d-tiles
NPAIR = NDT // 2         # 2 d-tile pairs
NOT = O // P             # 4 o-tiles
NTAPS = 5
N_CORES = 8
XW = CH + 4              # x tile width incl halos


def build_program():
    nc = bacc.Bacc()

    xrh = nc.declare_dram_parameter("xrh", [P, NCH * NDT * XW], BF16,
                                    isOutput=False)
    cwd = nc.declare_dram_parameter("cwd", [P, NDT * NTAPS * P], BF16,
                                    isOutput=False)
    hwTr = nc.declare_dram_parameter("hwTr", [P, NOT * NDT * P], BF16,
                                     isOutput=False)
    gw8d = nc.declare_dram_parameter("gw8", [P, NOT * NPAIR * 2 * P], F8E4,
                                     isOutput=False)
    outr = nc.declare_dram_parameter("outr", [P, NOT * L], BF16, isOutput=True)

    xr4 = xrh.rearrange("p (i dt l) -> p i dt l", dt=NDT, l=XW)
    outr3 = outr.rearrange("p (ot l) -> p ot l", l=L)

    with TileContext(nc) as tc:
        with (
            tc.tile_pool(name="weights", bufs=1) as wpool,
            tc.tile_pool(name="xin", bufs=3) as xpool,
            tc.tile_pool(name="csb", bufs=3) as cpool,
            tc.tile_pool(name="actout", bufs=4) as apool,
            tc.tile_pool(name="ztiles", bufs=3) as zpool,
            tc.tile_pool(name="vtiles", bufs=3) as vpool,
            tc.tile_pool(name="outt", bufs=3) as opool,
            tc.tile_pool(name="cps", bufs=2, space="PSUM") as cps_pool,
            tc.tile_pool(name="hps", bufs=2, space="PSUM") as hps_pool,
            tc.tile_pool(name="gps", bufs=2, space="PSUM") as gps_pool,
        ):
            # ---- PE warm-up: FULL-WIDTH dummy matmuls (128x128 x 512 cols).
            # Narrow warmups do not trip the HAM clock governor - it needs
            # sustained high array utilization - so burn max-power matmuls
            # until the first x/cw tiles land (~10us).
            warm_sb = wpool.tile([P, CH], BF16, tag="warm")
            nc.vector.memset(warm_sb, 0.0)
            wps = cps_pool.tile([P, 2, CH], F32, tag="cps", name="warmps")
            for _ in range(10):
                nc.tensor.matmul(wps[:, 0, :], lhsT=warm_sb[:, 0:P],
                                 rhs=warm_sb, start=True, stop=True)
            wout = wpool.tile([2, 2], F32, tag="warmout")
            nc.vector.tensor_copy(wout, wps[0:2, 0, 0:2])

            # ---- weight DMAs spread over the idle Vector/Scalar/GpSimd
            # HWDGE queues (x owns Sync) so the transfers run in parallel:
            # conv diagonals on Vector (conv(0) needs pair 0 first), h
            # halves on Scalar/GpSimd, fp8 g weights on Scalar.
            cw_sb = wpool.tile([P, NDT * NTAPS, P], BF16, tag="cw")
            CB = NTAPS * P
            gw8_sb = wpool.tile([P, NOT, NPAIR, 2, P], F8E4, tag="gw8")
            hwTr_sb = wpool.tile([P, NOT * NDT * P], BF16, tag="hwTr")
            HB = 2 * NDT * P  # columns per o-tile-pair block (1024)
            nc.gpsimd.dma_start(out=cw_sb[:, 0:2 * NTAPS, :],
                                in_=cwd[:, 0:2 * CB])
            nc.gpsimd.dma_start(out=cw_sb[:, 2 * NTAPS:4 * NTAPS, :],
                                in_=cwd[:, 2 * CB:4 * CB])
            nc.scalar.dma_start(out=gw8_sb, in_=gw8d[:, :])
            # the 1MB h-weight load is emitted later, gated on chunk-0's c,
            # so it doesn't contend with the critical x/cw startup DMA

            # ---- polarizing bias for o-tile 0 (rows 0/1 of g):
            # gbn0[p] = 1000*(p==0) - 1000*(p==1), built from a partition iota
            pidx = wpool.tile([P, 1], I32, tag="pidx")
            nc.gpsimd.iota(pidx, pattern=[[1, 1]], base=0, channel_multiplier=1)
            gbn0 = wpool.tile([P, 1], F32, tag="gbn0")
            gbn1 = wpool.tile([P, 1], F32, tag="gbn1")
            nc.gpsimd.tensor_scalar(gbn0, pidx, 0, 1000.0, OP.is_equal, OP.mult)
            nc.gpsimd.tensor_scalar(gbn1, pidx, 1, -1000.0, OP.is_equal, OP.mult)
            nc.gpsimd.tensor_tensor(gbn0, gbn0, gbn1, OP.add)
            # dummy sigmoid: pull the SIGMOID act-table load into the startup
            # window instead of the middle of the pipeline fill
            sigw = wpool.tile([P, 1], BF16, tag="sigw")
            nc.scalar.activation(sigw, gbn1, AF.Sigmoid, bias=0.0, scale=-1.0)

            c_sb = [None] * NCH       # [chunk] -> [128, 4, 512] bf16 tile
            c8_sb = [None] * NCH      # [chunk] -> [128, 2, 2, 512] fp8 tile
            prev_out = [None]         # previous chunk's big out tile

            conv_state = {}

            def emit_conv_mm(i):
                xt = xpool.tile([P, NDT, XW], BF16, tag="xt")
                if i <= 1:
                    # pair-granular arrival so each conv pair starts as soon
                    # as its own half of the chunk has landed
                    nc.sync.dma_start(out=xt[:, 0:2, :], in_=xr4[:, i, 0:2, :])
                    nc.sync.dma_start(out=xt[:, 2:4, :], in_=xr4[:, i, 2:4, :])
                else:
                    nc.sync.dma_start(out=xt, in_=xr4[:, i, :, :])
                tap_order = (2, 0, 1, 3, 4)
                cps = []
                for pr in range(NPAIR):
                    dtA, dtB = 2 * pr, 2 * pr + 1
                    cp = cps_pool.tile([P, 2, CH], F32, tag="cps",
                                       name=f"cp{i}_{pr}")
                    cps.append(cp)
                    for j, k in enumerate(tap_order):
                        st, sp = (j == 0), (j == NTAPS - 1)
                        # even d-tile: diagonal quadrants (0,0), (64,64)
                        nc.tensor.matmul(
                            cp[0:64, 0, :],
                            lhsT=cw_sb[0:64, dtA * NTAPS + k, 0:64],
                            rhs=xt[0:64, dtA, k:k + CH],
                            start=st, stop=sp, tile_position=(0, 0))
                        nc.tensor.matmul(
                            cp[64:128, 0, :],
                            lhsT=cw_sb[64:128, dtA * NTAPS + k, 64:128],
                            rhs=xt[64:128, dtA, k:k + CH],
                            start=st, stop=sp, tile_position=(64, 64))
                        # odd d-tile (x + weights partition-rolled by 64):
                        # anti-diagonal quadrants (0,64), (64,0)
                        nc.tensor.matmul(
                            cp[64:128, 1, :],
                            lhsT=cw_sb[0:64, dtB * NTAPS + k, 64:128],
                            rhs=xt[0:64, dtB, k:k + CH],
                            start=st, stop=sp, tile_position=(0, 64))
                        nc.tensor.matmul(
                            cp[0:64, 1, :],
                            lhsT=cw_sb[64:128, dtB * NTAPS + k, 0:64],
                            rhs=xt[64:128, dtB, k:k + CH],
                            start=st, stop=sp, tile_position=(64, 0))
                conv_state[i] = cps

            def emit_conv_copies(i):
                ct = cpool.tile([P, NDT, CH], BF16, tag="ct")
                c8t = cpool.tile([P, NPAIR, 2, CH], F8E4, tag="c8t")
                for pr in range(NPAIR):
                    cp = conv_state[i][pr]
                    # fp8 first: the g DoubleRow matmuls (emitted before h)
                    # depend on it, the bf16 copy only feeds the later h.
                    # During the pipeline fill DVE is idle - let it take the
                    # bf16 copies so psum recycles sooner.
                    nc.scalar.copy(c8t[:, pr, :, :], cp)
                    nc.scalar.copy(ct[:, 2 * pr:2 * pr + 2, :], cp)
                c_sb[i] = ct
                c8_sb[i] = c8t

            def emit_rest(i):
                ott = opool.tile([P, NOT, CH], BF16, tag="outt")
                last = i == NCH - 1
                halves = 2 if last else 1
                HW_ = CH // halves

                def emit_g(ot):
                    gp = gps_pool.tile([P, CH], F32, tag="gps")
                    for pr in range(NPAIR):
                        nc.tensor.matmul(
                            gp,
                            lhsT=gw8_sb[:, ot, pr, :, :],
                            rhs=c8_sb[i][:, pr, :, :],
                            start=(pr == 0), stop=(pr == NPAIR - 1),
                            perf_mode=PM.DoubleRow,
                        )
                    at = apool.tile([P, CH], BF16, tag="at")
                    zt = zpool.tile([P, CH], BF16, tag="zt")
                    for hf in range(halves):
                        sl = slice(hf * HW_, (hf + 1) * HW_)
                        nc.scalar.activation(at[:, sl], gp[:, sl], AF.Sigmoid,
                                             bias=(gbn0[:, :] if ot == 0
                                                   else 0.0),
                                             scale=-1.0)
                        # at the drain DVE's inline z shortens the relay;
                        # mid-stream Pool takes z to keep DVE de-saturated
                        zeng = nc.vector if last else nc.gpsimd
                        zeng.tensor_scalar(zt[:, sl], at[:, sl],
                                           -1.0, 1.0, OP.mult, OP.add)
                    return at, zt

                def emit_h(ot, at, zt):
                    if last and ot >= 2:
                        # g psum is done at the drain (all-g-first): the last
                        # two h matmuls take gps tiles so they never wait on
                        # hps recycling behind the DVE v-mult backlog
                        hp = gps_pool.tile([P, CH], F32, tag="gps",
                                           name=f"hx{ot}")
                    else:
                        hp = hps_pool.tile([P, CH], F32, tag="hps")
                    for dt in range(NDT):
                        nc.tensor.matmul(
                            hp,
                            lhsT=hwTr_sb[:, ot * 512 + dt * P:
                                         ot * 512 + dt * P + P],
                            rhs=c_sb[i][:, dt, :],
                            start=(dt == 0), stop=(dt == NDT - 1),
                        )
                    vt = vpool.tile([P, CH], BF16, tag="vt")
                    for hf in range(halves):
                        sl = slice(hf * HW_, (hf + 1) * HW_)
                        nc.vector.tensor_tensor(vt[:, sl], zt[:, sl],
                                                hp[:, sl], OP.mult)
                        if hf == 0:
                            init = (0.0 if i == 0
                                    else prev_out[0][:, ot, CH - 1:CH])
                        else:
                            init = ott[:, ot, hf * HW_ - 1:hf * HW_]
                        nc.vector.tensor_tensor_scan(
                            ott[:, ot, sl], at[:, sl], vt[:, sl], init,
                            OP.mult, OP.add)
                        if last:
                            # Sync is idle at the drain; keep Scalar's queue
                            # free for the sigmoid/z chain
                            nc.sync.dma_start(
                                out=outr3[:, ot, i * CH + hf * HW_:
                                          i * CH + (hf + 1) * HW_],
                                in_=ott[:, ot, sl])

                if i >= NCH - 2:
                    # all g matmuls first: every o-tile's sigmoid/z finishes
                    # while the h matmuls still stream, shortening the drain
                    az = [emit_g(ot) for ot in range(NOT)]
                    for ot in range(NOT):
                        emit_h(ot, *az[ot])
                else:
                    for ot in range(NOT):
                        at, zt = emit_g(ot)
                        emit_h(ot, at, zt)
                if not last:
                    # Sync queue: Scalar would head-of-line-block the conv
                    # psum->sbuf copies behind this store's scan dependency
                    nc.sync.dma_start(
                        out=outr3[:, :, i * CH:(i + 1) * CH], in_=ott)
                prev_out[0] = ott

            def emit_rest_last():
                # last chunk: column-split the g/h MATMULS too, so half-A's
                # v/scan chain runs on DVE while PE still streams half-B -
                # the post-PE DVE tail shrinks to one half-chunk's relay
                i = NCH - 1
                ott = opool.tile([P, NOT, CH], BF16, tag="outt")
                HW2 = CH // 2
                at_t, zt_t = {}, {}
                for ot in range(NOT):
                    at_t[ot] = apool.tile([P, CH], BF16, tag="at",
                                          name=f"at7_{ot}")
                    zt_t[ot] = zpool.tile([P, CH], BF16, tag="zt",
                                          name=f"zt7_{ot}")
                for hf in range(2):
                    sl = slice(hf * HW2, (hf + 1) * HW2)
                    for ot in range(NOT):
                        gp = gps_pool.tile([P, HW2], F32, tag="gps",
                                           name=f"g7_{ot}_{hf}")
                        for pr in range(NPAIR):
                            nc.tensor.matmul(
                                gp, lhsT=gw8_sb[:, ot, pr, :, :],
                                rhs=c8_sb[i][:, pr, :, sl],
                                start=(pr == 0), stop=(pr == NPAIR - 1),
                                perf_mode=PM.DoubleRow)
                        nc.scalar.activation(at_t[ot][:, sl], gp, AF.Sigmoid,
                                             bias=(gbn0[:, :] if ot == 0
                                                   else 0.0),
                                             scale=-1.0)
                        nc.vector.tensor_scalar(zt_t[ot][:, sl],
                                                at_t[ot][:, sl],
                                                -1.0, 1.0, OP.mult, OP.add)
                    for ot in range(NOT):
                        if ot >= 2:
                            hp = gps_pool.tile([P, HW2], F32, tag="gps",
                                               name=f"hx7_{ot}_{hf}")
                        else:
                            hp = hps_pool.tile([P, HW2], F32, tag="hps",
                                               name=f"h7_{ot}_{hf}")
                        for dt in range(NDT):
                            nc.tensor.matmul(
                                hp,
                                lhsT=hwTr_sb[:, ot * 512 + dt * P:
                                             ot * 512 + dt * P + P],
                                rhs=c_sb[i][:, dt, sl],
                                start=(dt == 0), stop=(dt == NDT - 1))
                        vt = vpool.tile([P, HW2], BF16, tag="vt",
                                        name=f"v7_{ot}_{hf}")
                        nc.vector.tensor_tensor(vt, zt_t[ot][:, sl], hp,
                                                OP.mult)
                        init = (prev_out[0][:, ot, CH - 1:CH] if hf == 0
                                else ott[:, ot, HW2 - 1:HW2])
                        nc.vector.tensor_tensor_scan(
                            ott[:, ot, sl], at_t[ot][:, sl], vt, init,
                            OP.mult, OP.add)
                        nc.sync.dma_start(
                            out=outr3[:, ot, i * CH + hf * HW2:
                                      i * CH + (hf + 1) * HW2],
                            in_=ott[:, ot, sl])
                prev_out[0] = ott

            # conv matmuls run two chunks ahead of rest; the psum->sbuf
            # copies are emitted separately so chunk i's sigmoids only sit
            # behind chunk i+1's copies in the Act queue (not i+1 and i+2)
            emit_conv_mm(0)
            emit_conv_copies(0)
            # gate bytes: the h-weight DMA triggers inherit a WAW dependency
            # on these copies (which wait for chunk-0's c), delaying the
            # transfer past the bandwidth-critical startup window. h weights
            # land ~18.5-20us, just ahead of rest(0)'s h matmuls.
            nc.vector.tensor_copy(hwTr_sb[0:1, 0:1], c_sb[0][0:1, 0, 0:1])
            nc.vector.tensor_copy(hwTr_sb[0:1, HB:HB + 1],
                                  c_sb[0][0:1, 0, 0:1])
            nc.gpsimd.dma_start(out=hwTr_sb[:, 0:HB], in_=hwTr[:, 0:HB])
            nc.gpsimd.dma_start(out=hwTr_sb[:, HB:2 * HB],
                                in_=hwTr[:, HB:2 * HB])
            emit_conv_mm(1)
            for i in range(NCH - 2):
                emit_rest(i)
                emit_conv_copies(i + 1)
                emit_conv_mm(i + 2)
            emit_rest(NCH - 2)
            emit_conv_copies(NCH - 1)
            emit_rest(NCH - 1)

    nc.finalize()
    return nc


_PROGRAM = None


def _get_program():
    global _PROGRAM
    if _PROGRAM is None:
        _PROGRAM = build_program()
    return _PROGRAM


def prepare_in_maps(x, conv_w, h_w, g_w):
    x = np.ascontiguousarray(np.asarray(x), dtype=np.float32)
    conv_w = np.asarray(conv_w, dtype=np.float32)
    h_w = np.asarray(h_w, dtype=np.float32)
    g_w = np.asarray(g_w, dtype=np.float32)

    # hwTr[p, ot*512 + dt*128 + m] = h_w[ot*128+m, dt*128+p]  (bf16)
    hw = h_w[:, :, 0]                                             # [O, D]
    hwTr = np.ascontiguousarray(
        hw.reshape(NOT, P, NDT, P).transpose(3, 0, 2, 1).reshape(P, -1)
        .astype(ml_dtypes.bfloat16))

    # gw8[p, ot, pair, j, m] = gwp[ot*128+m, (2*pair+j)*128+p]  (fp8 e4m3)
    gwp = np.zeros((O, D), np.float32)
    gwp[2:, :] = g_w[:, :, 0]
    gw8 = np.ascontiguousarray(
        gwp.reshape(NOT, P, NPAIR, 2, P).transpose(4, 0, 2, 3, 1)
        .reshape(P, -1).astype(ml_dtypes.float8_e4m3))

    # cwd[p, (dt*5+k)*128 + q] = (q == p) * conv_w[dt*128+p, 0, k], with the
    # odd d-tiles' diagonal rolled by 64 partitions (matmul tile packing).
    cwd = np.zeros((P, NDT * NTAPS, P), np.float32)
    q = np.arange(P)
    for dt in range(NDT):
        pq = (q + 64) % P if dt % 2 else q
        for k in range(NTAPS):
            cwd[pq, dt * NTAPS + k, q] = conv_w[dt * P + q, 0, k]
    cwd = np.ascontiguousarray(cwd.reshape(P, -1).astype(ml_dtypes.bfloat16))

    # xrh[p, i, dt, j] = xpad[dt*128+p, i*512 + j] with 2-col zero halos;
    # odd d-tiles partition-rolled by 64.
    in_maps = []
    for b in range(B):
        xpad = np.pad(x[b], ((0, 0), (2, 2))).astype(ml_dtypes.bfloat16)
        xpad = xpad.reshape(NDT, P, L + 4)
        xroll = np.empty_like(xpad)
        for dt in range(NDT):
            xroll[dt] = np.roll(xpad[dt], 64, axis=0) if dt % 2 else xpad[dt]
        xb = np.empty((P, NCH, NDT, XW), dtype=ml_dtypes.bfloat16)
        for i in range(NCH):
            xb[:, i, :, :] = xroll[:, :, i * CH:i * CH + XW].transpose(1, 0, 2)
        in_maps.append({"xrh": np.ascontiguousarray(xb.reshape(P, -1)),
                        "cwd": cwd, "hwTr": hwTr, "gw8": gw8})
    return in_maps


def kernel(x, conv_w, h_w, g_w):
    in_maps = prepare_in_maps(x, conv_w, h_w, g_w)
    nc = _get_program()
    res = run_bass_kernel_spmd(nc, in_maps, list(range(N_CORES))).results
    return np.stack(
        [res[b]["outr"].reshape(P, NOT, L).transpose(1, 0, 2).reshape(O, L)
         .astype(np.float32)
         for b in range(B)], axis=0)


# revision 75
# speedup vs baseline: 1.1720x; 1.1720x over previous
"""MinGRU Trainium2 kernel (v4).

Reference computation (per batch b):
    c = depthwise_conv1d(x, conv_w, taps=5, pad=2)        # [D, L]
    h = h_w @ c                                           # [O, L]
    g = concat([-1000, +1000], g_w @ c)                   # [O, L]
    a = sigmoid(-g); v = (1 - a) * h
    out[l] = a[l] * out[l-1] + v[l]     (linear scan along L)

Strategy: pure data-parallel over B (8 batches -> 8 NeuronCores).
Per core, stream in l-chunks of 512:
  - conv: diagonal matmuls on TensorE, packed as 64x64 array tiles: a
    d-tile PAIR runs 4 concurrent tile-position matmuls per tap
    ((0,0),(64,64) for the even d-tile; (0,64),(64,0) for the odd one,
    whose x/weights are partition-rolled by 64 host-side).  Measured
    1.68x over full-width diagonal matmuls.
  - c PSUM->SBUF copies on ScalarE; h/g 1x1-conv matmuls bf16.
  - a = sigmoid(-(g + bias)) on ScalarE (bias carries +/-1000 polarized
    rows 0/1, built on-chip from a partition iota); z = 1 - a on GpSimd;
    v = z*h on VectorE; scan via tensor_tensor_scan (DVE).
  - rows 0/1 come out of the scan naturally (a saturates to exactly 1/0).
  - x is relaid out host-side to [128, chunk, dt, 516] WITH the +/-2
    halos baked in (edges zero-padded), odd d-tiles rolled by 64
    partitions, so each chunk loads with ONE contiguous DMA.
  - out is bf16 (host converts to f32; ~0.2% extra rounding, well under
    the 2e-2 gate), written as [128, (ot, chunk, col)] and permuted
    host-side; chunk 7 runs its sigmoid/z/v/scan chain on 256-col halves
    with per-o-tile stores to shorten the tail.
"""

import numpy as np
import ml_dtypes

import concourse.bass as bass
import concourse.mybir as mybir
from concourse import bacc
from concourse.tile import TileContext
from concourse.bass_utils import run_bass_kernel_spmd

F32 = mybir.dt.float32
BF16 = mybir.dt.bfloat16
F8E4 = mybir.dt.float8e4
I32 = mybir.dt.int32
AF = mybir.ActivationFunctionType
OP = mybir.AluOpType
PM = mybir.MatmulPerfMode

B, D, O, L = 8, 512, 512, 4096
P = 128
CH = 512                 # l-chunk width (one PSUM bank)
NCH = L // CH            # 8
NDT = D // P             # 4 d-tiles
NPAIR = NDT // 2         # 2 d-tile pairs
NOT = O // P             # 4 o-tiles
NTAPS = 5
N_CORES = 8
XW = CH + 4              # x tile width incl halos


def build_program():
    nc = bacc.Bacc()

    xrh = nc.declare_dram_parameter("xrh", [P, NCH * NDT * XW], BF16,
                                    isOutput=False)
    cwd = nc.declare_dram_parameter("cwd", [P, NDT * NTAPS * P], BF16,
                                    isOutput=False)
    hwTr = nc.declare_dram_parameter("hwTr", [P, NOT * NDT * P], BF16,
                                     isOutput=False)
    gw8d = nc.declare_dram_parameter("gw8", [P, NOT * NPAIR * 2 * P], F8E4,
                                     isOutput=False)
    outr = nc.declare_dram_parameter("outr", [P, NOT * L], BF16, isOutput=True)

    xr4 = xrh.rearrange("p (i dt l) -> p i dt l", dt=NDT, l=XW)
    outr3 = outr.rearrange("p (ot l) -> p ot l", l=L)

    with TileContext(nc) as tc:
        with (
            tc.tile_pool(name="weights", bufs=1) as wpool,
            tc.tile_pool(name="xin", bufs=3) as xpool,
            tc.tile_pool(name="csb", bufs=3) as cpool,
            tc.tile_pool(name="actout", bufs=4) as apool,
            tc.tile_pool(name="ztiles", bufs=3) as zpool,
            tc.tile_pool(name="vtiles", bufs=3) as vpool,
            tc.tile_pool(name="outt", bufs=3) as opool,
            tc.tile_pool(name="cps", bufs=2, space="PSUM") as cps_pool,
            tc.tile_pool(name="hps", bufs=2, space="PSUM") as hps_pool,
            tc.tile_pool(name="gps", bufs=2, space="PSUM") as gps_pool,
        ):
            # ---- PE warm-up: FULL-WIDTH dummy matmuls (128x128 x 512 cols).
            # Narrow warmups do not trip the HAM clock governor - it needs
            # sustained high array utilization - so burn max-power matmuls
            # until the first x/cw tiles land (~10us).
            warm_sb = wpool.tile([P, CH], BF16, tag="warm")
            nc.vector.memset(warm_sb, 0.0)
            wps = cps_pool.tile([P, 2, CH], F32, tag="cps", name="warmps")
            for _ in range(10):
                nc.tensor.matmul(wps[:, 0, :], lhsT=warm_sb[:, 0:P],
                                 rhs=warm_sb, start=True, stop=True)
            wout = wpool.tile([2, 2], F32, tag="warmout")
            nc.vector.tensor_copy(wout, wps[0:2, 0, 0:2])

            # ---- weight DMAs spread over the idle Vector/Scalar/GpSimd
            # HWDGE queues (x owns Sync) so the transfers run in parallel:
            # conv diagonals on Vector (conv(0) needs pair 0 first), h
            # halves on Scalar/GpSimd, fp8 g weights on Scalar.
            cw_sb = wpool.tile([P, NDT * NTAPS, P], BF16, tag="cw")
            CB = NTAPS * P
            gw8_sb = wpool.tile([P, NOT, NPAIR, 2, P], F8E4, tag="gw8")
            hwTr_sb = wpool.tile([P, NOT * NDT * P], BF16, tag="hwTr")
            HB = 2 * NDT * P  # columns per o-tile-pair block (1024)
            nc.gpsimd.dma_start(out=cw_sb[:, 0:2 * NTAPS, :],
                                in_=cwd[:, 0:2 * CB])
            nc.gpsimd.dma_start(out=cw_sb[:, 2 * NTAPS:4 * NTAPS, :],
                                in_=cwd[:, 2 * CB:4 * CB])
            nc.scalar.dma_start(out=hwTr_sb[:, 0:HB], in_=hwTr[:, 0:HB])
            nc.scalar.dma_start(out=gw8_sb, in_=gw8d[:, :])
            nc.gpsimd.dma_start(out=hwTr_sb[:, HB:2 * HB],
                                in_=hwTr[:, HB:2 * HB])

            # ---- polarizing bias for o-tile 0 (rows 0/1 of g):
            # gbn0[p] = 1000*(p==0) - 1000*(p==1), built from a partition iota
            pidx = wpool.tile([P, 1], I32, tag="pidx")
            nc.gpsimd.iota(pidx, pattern=[[1, 1]], base=0, channel_multiplier=1)
            gbn0 = wpool.tile([P, 1], F32, tag="gbn0")
            gbn1 = wpool.tile([P, 1], F32, tag="gbn1")
            nc.gpsimd.tensor_scalar(gbn0, pidx, 0, 1000.0, OP.is_equal, OP.mult)
            nc.gpsimd.tensor_scalar(gbn1, pidx, 1, -1000.0, OP.is_equal, OP.mult)
            nc.gpsimd.tensor_tensor(gbn0, gbn0, gbn1, OP.add)
            # dummy sigmoid: pull the SIGMOID act-table load into the startup
            # window instead of the middle of the pipeline fill
            sigw = wpool.tile([P, 1], BF16, tag="sigw")
            nc.scalar.activation(sigw, gbn1, AF.Sigmoid, bias=0.0, scale=-1.0)

            c_sb = [None] * NCH       # [chunk] -> [128, 4, 512] bf16 tile
            c8_sb = [None] * NCH      # [chunk] -> [128, 2, 2, 512] fp8 tile
            prev_out = [None]         # previous chunk's big out tile

            conv_state = {}

            def emit_conv_mm(i):
                xt = xpool.tile([P, NDT, XW], BF16, tag="xt")
                if i <= 1:
                    # pair-granular arrival so each conv pair starts as soon
                    # as its own half of the chunk has landed
                    nc.sync.dma_start(out=xt[:, 0:2, :], in_=xr4[:, i, 0:2, :])
                    nc.sync.dma_start(out=xt[:, 2:4, :], in_=xr4[:, i, 2:4, :])
                else:
                    nc.sync.dma_start(out=xt, in_=xr4[:, i, :, :])
                tap_order = (2, 0, 1, 3, 4)
                cps = []
                for pr in range(NPAIR):
                    dtA, dtB = 2 * pr, 2 * pr + 1
                    cp = cps_pool.tile([P, 2, CH], F32, tag="cps",
                                       name=f"cp{i}_{pr}")
                    cps.append(cp)
                    for j, k in enumerate(tap_order):
                        st, sp = (j == 0), (j == NTAPS - 1)
                        # even d-tile: diagonal quadrants (0,0), (64,64)
                        nc.tensor.matmul(
                            cp[0:64, 0, :],
                            lhsT=cw_sb[0:64, dtA * NTAPS + k, 0:64],
                            rhs=xt[0:64, dtA, k:k + CH],
                            start=st, stop=sp, tile_position=(0, 0))
                        nc.tensor.matmul(
                            cp[64:128, 0, :],
                            lhsT=cw_sb[64:128, dtA * NTAPS + k, 64:128],
                            rhs=xt[64:128, dtA, k:k + CH],
                            start=st, stop=sp, tile_position=(64, 64))
                        # odd d-tile (x + weights partition-rolled by 64):
                        # anti-diagonal quadrants (0,64), (64,0)
                        nc.tensor.matmul(
                            cp[64:128, 1, :],
                            lhsT=cw_sb[0:64, dtB * NTAPS + k, 64:128],
                            rhs=xt[0:64, dtB, k:k + CH],
                            start=st, stop=sp, tile_position=(0, 64))
                        nc.tensor.matmul(
                            cp[0:64, 1, :],
                            lhsT=cw_sb[64:128, dtB * NTAPS + k, 0:64],
                            rhs=xt[64:128, dtB, k:k + CH],
                            start=st, stop=sp, tile_position=(64, 0))
                conv_state[i] = cps

            def emit_conv_copies(i):
                ct = cpool.tile([P, NDT, CH], BF16, tag="ct")
                c8t = cpool.tile([P, NPAIR, 2, CH], F8E4, tag="c8t")
                for pr in range(NPAIR):
                    cp = conv_state[i][pr]
                    # fp8 first: the g DoubleRow matmuls (emitted before h)
                    # depend on it, the bf16 copy only feeds the later h.
                    # During the pipeline fill DVE is idle - let it take the
                    # bf16 copies so psum recycles sooner.
                    nc.scalar.copy(c8t[:, pr, :, :], cp)
                    nc.scalar.copy(ct[:, 2 * pr:2 * pr + 2, :], cp)
                c_sb[i] = ct
                c8_sb[i] = c8t

            def emit_rest(i):
                ott = opool.tile([P, NOT, CH], BF16, tag="outt")
                last = i == NCH - 1
                halves = 2 if last else 1
                HW_ = CH // halves

                def emit_g(ot):
                    gp = gps_pool.tile([P, CH], F32, tag="gps")
                    for pr in range(NPAIR):
                        nc.tensor.matmul(
                            gp,
                            lhsT=gw8_sb[:, ot, pr, :, :],
                            rhs=c8_sb[i][:, pr, :, :],
                            start=(pr == 0), stop=(pr == NPAIR - 1),
                            perf_mode=PM.DoubleRow,
                        )
                    at = apool.tile([P, CH], BF16, tag="at")
                    zt = zpool.tile([P, CH], BF16, tag="zt")
                    for hf in range(halves):
                        sl = slice(hf * HW_, (hf + 1) * HW_)
                        nc.scalar.activation(at[:, sl], gp[:, sl], AF.Sigmoid,
                                             bias=(gbn0[:, :] if ot == 0
                                                   else 0.0),
                                             scale=-1.0)
                        # at the drain DVE's inline z shortens the relay;
                        # mid-stream Pool takes z to keep DVE de-saturated
                        zeng = nc.vector if last else nc.gpsimd
                        zeng.tensor_scalar(zt[:, sl], at[:, sl],
                                           -1.0, 1.0, OP.mult, OP.add)
                    return at, zt

                def emit_h(ot, at, zt):
                    if last and ot >= 2:
                        # g psum is done at the drain (all-g-first): the last
                        # two h matmuls take gps tiles so they never wait on
                        # hps recycling behind the DVE v-mult backlog
                        hp = gps_pool.tile([P, CH], F32, tag="gps",
                                           name=f"hx{ot}")
                    else:
                        hp = hps_pool.tile([P, CH], F32, tag="hps")
                    for dt in range(NDT):
                        nc.tensor.matmul(
                            hp,
                            lhsT=hwTr_sb[:, ot * 512 + dt * P:
                                         ot * 512 + dt * P + P],
                            rhs=c_sb[i][:, dt, :],
                            start=(dt == 0), stop=(dt == NDT - 1),
                        )
                    vt = vpool.tile([P, CH], BF16, tag="vt")
                    for hf in range(halves):
                        sl = slice(hf * HW_, (hf + 1) * HW_)
                        nc.vector.tensor_tensor(vt[:, sl], zt[:, sl],
                                                hp[:, sl], OP.mult)
                        if hf == 0:
                            init = (0.0 if i == 0
                                    else prev_out[0][:, ot, CH - 1:CH])
                        else:
                            init = ott[:, ot, hf * HW_ - 1:hf * HW_]
                        nc.vector.tensor_tensor_scan(
                            ott[:, ot, sl], at[:, sl], vt[:, sl], init,
                            OP.mult, OP.add)
                        if last:
                            # Sync is idle at the drain; keep Scalar's queue
                            # free for the sigmoid/z chain
                            nc.sync.dma_start(
                                out=outr3[:, ot, i * CH + hf * HW_:
                                          i * CH + (hf + 1) * HW_],
                                in_=ott[:, ot, sl])

                if i >= NCH - 2:
                    # all g matmuls first: every o-tile's sigmoid/z finishes
                    # while the h matmuls still stream, shortening the drain
                    az = [emit_g(ot) for ot in range(NOT)]
                    for ot in range(NOT):
                        emit_h(ot, *az[ot])
                else:
                    for ot in range(NOT):
                        at, zt = emit_g(ot)
                        emit_h(ot, at, zt)
                if not last:
                    # Sync queue: Scalar would head-of-line-block the conv
                    # psum->sbuf copies behind this store's scan dependency
                    nc.sync.dma_start(
                        out=outr3[:, :, i * CH:(i + 1) * CH], in_=ott)
                prev_out[0] = ott

            def emit_rest_last():
                # last chunk: column-split the g/h MATMULS too, so half-A's
                # v/scan chain runs on DVE while PE still streams half-B -
                # the post-PE DVE tail shrinks to one half-chunk's relay
                i = NCH - 1
                ott = opool.tile([P, NOT, CH], BF16, tag="outt")
                HW2 = CH // 2
                at_t, zt_t = {}, {}
                for ot in range(NOT):
                    at_t[ot] = apool.tile([P, CH], BF16, tag="at",
                                          name=f"at7_{ot}")
                    zt_t[ot] = zpool.tile([P, CH], BF16, tag="zt",
                                          name=f"zt7_{ot}")
                for hf in range(2):
                    sl = slice(hf * HW2, (hf + 1) * HW2)
                    for ot in range(NOT):
                        gp = gps_pool.tile([P, HW2], F32, tag="gps",
                                           name=f"g7_{ot}_{hf}")
                        for pr in range(NPAIR):
                            nc.tensor.matmul(
                                gp, lhsT=gw8_sb[:, ot, pr, :, :],
                                rhs=c8_sb[i][:, pr, :, sl],
                                start=(pr == 0), stop=(pr == NPAIR - 1),
                                perf_mode=PM.DoubleRow)
                        nc.scalar.activation(at_t[ot][:, sl], gp, AF.Sigmoid,
                                             bias=(gbn0[:, :] if ot == 0
                                                   else 0.0),
                                             scale=-1.0)
                        nc.vector.tensor_scalar(zt_t[ot][:, sl],
                                                at_t[ot][:, sl],
                                                -1.0, 1.0, OP.mult, OP.add)
                    for ot in range(NOT):
                        if ot >= 2:
                            hp = gps_pool.tile([P, HW2], F32, tag="gps",
                                               name=f"hx7_{ot}_{hf}")
                        else:
                            hp = hps_pool.tile([P, HW2], F32, tag="hps",
                                               name=f"h7_{ot}_{hf}")
                        for dt in range(NDT):
                            nc.tensor.matmul(
                                hp,
                                lhsT=hwTr_sb[:, ot * 512 + dt * P:
                                             ot * 512 + dt * P + P],
                                rhs=c_sb[i][:, dt, sl],
                                start=(dt == 0), stop=(dt == NDT - 1))
                        vt = vpool.tile([P, HW2], BF16, tag="vt",
                                        name=f"v7_{ot}_{hf}")
                        nc.vector.tensor_tensor(vt, zt_t[ot][:, sl], hp,
                                                OP.mult)
                        init = (prev_out[0][:, ot, CH - 1:CH] if hf == 0
                                else ott[:, ot, HW2 - 1:HW2])
                        nc.vector.tensor_tensor_scan(
                            ott[:, ot, sl], at_t[ot][:, sl], vt, init,
                            OP.mult, OP.add)
                        nc.sync.dma_start(
                            out=outr3[:, ot, i * CH + hf * HW2:
                                      i * CH + (hf + 1) * HW2],
                            in_=ott[:, ot, sl])
                prev_out[0] = ott

            # conv matmuls run two chunks ahead of rest; the psum->sbuf
            # copies are emitted separately so chunk i's sigmoids only sit
            # behind chunk i+1's copies in the Act queue (not i+1 and i+2)
            emit_conv_mm(0)
            emit_conv_copies(0)
            emit_conv_mm(1)
            for i in range(NCH - 2):
                emit_rest(i)
                emit_conv_copies(i + 1)
                emit_conv_mm(i + 2)
            emit_rest(NCH - 2)
            emit_conv_copies(NCH - 1)
            emit_rest(NCH - 1)

    nc.finalize()
    return nc


_PROGRAM = None


def _get_program():
    global _PROGRAM
    if _PROGRAM is None:
        _PROGRAM = build_program()
    return _PROGRAM


def prepare_in_maps(x, conv_w, h_w, g_w):
    x = np.ascontiguousarray(np.asarray(x), dtype=np.float32)
    conv_w = np.asarray(conv_w, dtype=np.float32)
    h_w = np.asarray(h_w, dtype=np.float32)
    g_w = np.asarray(g_w, dtype=np.float32)

    # hwTr[p, ot*512 + dt*128 + m] = h_w[ot*128+m, dt*128+p]  (bf16)
    hw = h_w[:, :, 0]                                             # [O, D]
    hwTr = np.ascontiguousarray(
        hw.reshape(NOT, P, NDT, P).transpose(3, 0, 2, 1).reshape(P, -1)
        .astype(ml_dtypes.bfloat16))

    # gw8[p, ot, pair, j, m] = gwp[ot*128+m, (2*pair+j)*128+p]  (fp8 e4m3)
    gwp = np.zeros((O, D), np.float32)
    gwp[2:, :] = g_w[:, :, 0]
    gw8 = np.ascontiguousarray(
        gwp.reshape(NOT, P, NPAIR, 2, P).transpose(4, 0, 2, 3, 1)
        .reshape(P, -1).astype(ml_dtypes.float8_e4m3))

    # cwd[p, (dt*5+k)*128 + q] = (q == p) * conv_w[dt*128+p, 0, k], with the
    # odd d-tiles' diagonal rolled by 64 partitions (matmul tile packing).
    cwd = np.zeros((P, NDT * NTAPS, P), np.float32)
    q = np.arange(P)
    for dt in range(NDT):
        pq = (q + 64) % P if dt % 2 else q
        for k in range(NTAPS):
            cwd[pq, dt * NTAPS + k, q] = conv_w[dt * P + q, 0, k]
    cwd = np.ascontiguousarray(cwd.reshape(P, -1).astype(ml_dtypes.bfloat16))

    # xrh[p, i, dt, j] = xpad[dt*128+p, i*512 + j] with 2-col zero halos;
    # odd d-tiles partition-rolled by 64.
    in_maps = []
    for b in range(B):
        xpad = np.pad(x[b], ((0, 0), (2, 2))).astype(ml_dtypes.bfloat16)
        xpad = xpad.reshape(NDT, P, L + 4)
        xroll = np.empty_like(xpad)
        for dt in range(NDT):
            xroll[dt] = np.roll(xpad[dt], 64, axis=0) if dt % 2 else xpad[dt]
        xb = np.empty((P, NCH, NDT, XW), dtype=ml_dtypes.bfloat16)
        for i in range(NCH):
            xb[:, i, :, :] = xroll[:, :, i * CH:i * CH + XW].transpose(1, 0, 2)
        in_maps.append({"xrh": np.ascontiguousarray(xb.reshape(P, -1)),
                        "cwd": cwd, "hwTr": hwTr, "gw8": gw8})
    return in_maps


def kernel(x, conv_w, h_w, g_w):
    in_maps = prepare_in_maps(x, conv_w, h_w, g_w)
    nc = _get_program()
    res = run_bass_kernel_spmd(nc, in_maps, list(range(N_CORES))).results
    return np.stack(
        [res[b]["outr"].reshape(P, NOT, L).transpose(1, 0, 2).reshape(O, L)
         .astype(np.float32)
         for b in range(B)], axis=0)


# revision 76
# speedup vs baseline: 1.1782x; 1.0053x over previous
"""MinGRU Trainium2 kernel (v4).

Reference computation (per batch b):
    c = depthwise_conv1d(x, conv_w, taps=5, pad=2)        # [D, L]
    h = h_w @ c                                           # [O, L]
    g = concat([-1000, +1000], g_w @ c)                   # [O, L]
    a = sigmoid(-g); v = (1 - a) * h
    out[l] = a[l] * out[l-1] + v[l]     (linear scan along L)

Strategy: pure data-parallel over B (8 batches -> 8 NeuronCores).
Per core, stream in l-chunks of 512:
  - conv: diagonal matmuls on TensorE, packed as 64x64 array tiles: a
    d-tile PAIR runs 4 concurrent tile-position matmuls per tap
    ((0,0),(64,64) for the even d-tile; (0,64),(64,0) for the odd one,
    whose x/weights are partition-rolled by 64 host-side).  Measured
    1.68x over full-width diagonal matmuls.
  - c PSUM->SBUF copies on ScalarE; h/g 1x1-conv matmuls bf16.
  - a = sigmoid(-(g + bias)) on ScalarE (bias carries +/-1000 polarized
    rows 0/1, built on-chip from a partition iota); z = 1 - a on GpSimd;
    v = z*h on VectorE; scan via tensor_tensor_scan (DVE).
  - rows 0/1 come out of the scan naturally (a saturates to exactly 1/0).
  - x is relaid out host-side to [128, chunk, dt, 516] WITH the +/-2
    halos baked in (edges zero-padded), odd d-tiles rolled by 64
    partitions, so each chunk loads with ONE contiguous DMA.
  - out is bf16 (host converts to f32; ~0.2% extra rounding, well under
    the 2e-2 gate), written as [128, (ot, chunk, col)] and permuted
    host-side; chunk 7 runs its sigmoid/z/v/scan chain on 256-col halves
    with per-o-tile stores to shorten the tail.
"""

import numpy as np
import ml_dtypes

import concourse.bass as bass
import concourse.mybir as mybir
from concourse import bacc
from concourse.tile import TileContext
from concourse.bass_utils import run_bass_kernel_spmd

F32 = mybir.dt.float32
BF16 = mybir.dt.bfloat16
F8E4 = mybir.dt.float8e4
I32 = mybir.dt.int32
AF = mybir.ActivationFunctionType
OP = mybir.AluOpType
PM = mybir.MatmulPerfMode

B, D, O, L = 8, 512, 512, 4096
P = 128
CH = 512                 # l-chunk width (one PSUM bank)
NCH = L // CH            # 8
NDT = D // P             # 4 d-tiles
NPAIR = NDT // 2         # 2 d-tile pairs
NOT = O // P             # 4 o-tiles
NTAPS = 5
N_CORES = 8
XW = CH + 4              # x tile width incl halos


def build_program():
    nc = bacc.Bacc()

    xrh = nc.declare_dram_parameter("xrh", [P, NCH * NDT * XW], BF16,
                                    isOutput=False)
    cwd = nc.declare_dram_parameter("cwd", [P, NDT * NTAPS * P], BF16,
                                    isOutput=False)
    hwTr = nc.declare_dram_parameter("hwTr", [P, NOT * NDT * P], BF16,
                                     isOutput=False)
    gw8d = nc.declare_dram_parameter("gw8", [P, NOT * NPAIR * 2 * P], F8E4,
                                     isOutput=False)
    outr = nc.declare_dram_parameter("outr", [P, NOT * L], BF16, isOutput=True)

    xr4 = xrh.rearrange("p (i dt l) -> p i dt l", dt=NDT, l=XW)
    outr3 = outr.rearrange("p (ot l) -> p ot l", l=L)

    with TileContext(nc) as tc:
        with (
            tc.tile_pool(name="weights", bufs=1) as wpool,
            tc.tile_pool(name="xin", bufs=3) as xpool,
            tc.tile_pool(name="csb", bufs=3) as cpool,
            tc.tile_pool(name="actout", bufs=4) as apool,
            tc.tile_pool(name="ztiles", bufs=3) as zpool,
            tc.tile_pool(name="vtiles", bufs=3) as vpool,
            tc.tile_pool(name="outt", bufs=3) as opool,
            tc.tile_pool(name="cps", bufs=2, space="PSUM") as cps_pool,
            tc.tile_pool(name="hps", bufs=2, space="PSUM") as hps_pool,
            tc.tile_pool(name="gps", bufs=2, space="PSUM") as gps_pool,
        ):
            # ---- PE warm-up: FULL-WIDTH dummy matmuls (128x128 x 512 cols).
            # Narrow warmups do not trip the HAM clock governor - it needs
            # sustained high array utilization - so burn max-power matmuls
            # until the first x/cw tiles land (~10us).
            warm_sb = wpool.tile([P, CH], BF16, tag="warm")
            nc.vector.memset(warm_sb, 0.0)
            wps = cps_pool.tile([P, 2, CH], F32, tag="cps", name="warmps")
            for _ in range(12):
                nc.tensor.matmul(wps[:, 0, :], lhsT=warm_sb[:, 0:P],
                                 rhs=warm_sb, start=True, stop=True)
            wout = wpool.tile([2, 2], F32, tag="warmout")
            nc.vector.tensor_copy(wout, wps[0:2, 0, 0:2])

            # ---- weight DMAs spread over the idle Vector/Scalar/GpSimd
            # HWDGE queues (x owns Sync) so the transfers run in parallel:
            # conv diagonals on Vector (conv(0) needs pair 0 first), h
            # halves on Scalar/GpSimd, fp8 g weights on Scalar.
            cw_sb = wpool.tile([P, NDT * NTAPS, P], BF16, tag="cw")
            CB = NTAPS * P
            gw8_sb = wpool.tile([P, NOT, NPAIR, 2, P], F8E4, tag="gw8")
            hwTr_sb = wpool.tile([P, NOT * NDT * P], BF16, tag="hwTr")
            HB = 2 * NDT * P  # columns per o-tile-pair block (1024)
            nc.gpsimd.dma_start(out=cw_sb[:, 0:2 * NTAPS, :],
                                in_=cwd[:, 0:2 * CB])
            nc.gpsimd.dma_start(out=cw_sb[:, 2 * NTAPS:4 * NTAPS, :],
                                in_=cwd[:, 2 * CB:4 * CB])
            nc.scalar.dma_start(out=hwTr_sb[:, 0:HB], in_=hwTr[:, 0:HB])
            nc.scalar.dma_start(out=gw8_sb, in_=gw8d[:, :])
            nc.gpsimd.dma_start(out=hwTr_sb[:, HB:2 * HB],
                                in_=hwTr[:, HB:2 * HB])

            # ---- polarizing bias for o-tile 0 (rows 0/1 of g):
            # gbn0[p] = 1000*(p==0) - 1000*(p==1), built from a partition iota
            pidx = wpool.tile([P, 1], I32, tag="pidx")
            nc.gpsimd.iota(pidx, pattern=[[1, 1]], base=0, channel_multiplier=1)
            gbn0 = wpool.tile([P, 1], F32, tag="gbn0")
            gbn1 = wpool.tile([P, 1], F32, tag="gbn1")
            nc.gpsimd.tensor_scalar(gbn0, pidx, 0, 1000.0, OP.is_equal, OP.mult)
            nc.gpsimd.tensor_scalar(gbn1, pidx, 1, -1000.0, OP.is_equal, OP.mult)
            nc.gpsimd.tensor_tensor(gbn0, gbn0, gbn1, OP.add)
            # dummy sigmoid: pull the SIGMOID act-table load into the startup
            # window instead of the middle of the pipeline fill
            sigw = wpool.tile([P, 1], BF16, tag="sigw")
            nc.scalar.activation(sigw, gbn1, AF.Sigmoid, bias=0.0, scale=-1.0)

            c_sb = [None] * NCH       # [chunk] -> [128, 4, 512] bf16 tile
            c8_sb = [None] * NCH      # [chunk] -> [128, 2, 2, 512] fp8 tile
            prev_out = [None]         # previous chunk's big out tile

            conv_state = {}

            def emit_conv_mm(i):
                xt = xpool.tile([P, NDT, XW], BF16, tag="xt")
                if i <= 1:
                    # pair-granular arrival so each conv pair starts as soon
                    # as its own half of the chunk has landed
                    nc.sync.dma_start(out=xt[:, 0:2, :], in_=xr4[:, i, 0:2, :])
                    nc.sync.dma_start(out=xt[:, 2:4, :], in_=xr4[:, i, 2:4, :])
                else:
                    nc.sync.dma_start(out=xt, in_=xr4[:, i, :, :])
                tap_order = (2, 0, 1, 3, 4)
                cps = []
                for pr in range(NPAIR):
                    dtA, dtB = 2 * pr, 2 * pr + 1
                    cp = cps_pool.tile([P, 2, CH], F32, tag="cps",
                                       name=f"cp{i}_{pr}")
                    cps.append(cp)
                    for j, k in enumerate(tap_order):
                        st, sp = (j == 0), (j == NTAPS - 1)
                        # even d-tile: diagonal quadrants (0,0), (64,64)
                        nc.tensor.matmul(
                            cp[0:64, 0, :],
                            lhsT=cw_sb[0:64, dtA * NTAPS + k, 0:64],
                            rhs=xt[0:64, dtA, k:k + CH],
                            start=st, stop=sp, tile_position=(0, 0))
                        nc.tensor.matmul(
                            cp[64:128, 0, :],
                            lhsT=cw_sb[64:128, dtA * NTAPS + k, 64:128],
                            rhs=xt[64:128, dtA, k:k + CH],
                            start=st, stop=sp, tile_position=(64, 64))
                        # odd d-tile (x + weights partition-rolled by 64):
                        # anti-diagonal quadrants (0,64), (64,0)
                        nc.tensor.matmul(
                            cp[64:128, 1, :],
                            lhsT=cw_sb[0:64, dtB * NTAPS + k, 64:128],
                            rhs=xt[0:64, dtB, k:k + CH],
                            start=st, stop=sp, tile_position=(0, 64))
                        nc.tensor.matmul(
                            cp[0:64, 1, :],
                            lhsT=cw_sb[64:128, dtB * NTAPS + k, 0:64],
                            rhs=xt[64:128, dtB, k:k + CH],
                            start=st, stop=sp, tile_position=(64, 0))
                conv_state[i] = cps

            def emit_conv_copies(i):
                ct = cpool.tile([P, NDT, CH], BF16, tag="ct")
                c8t = cpool.tile([P, NPAIR, 2, CH], F8E4, tag="c8t")
                for pr in range(NPAIR):
                    cp = conv_state[i][pr]
                    # fp8 first: the g DoubleRow matmuls (emitted before h)
                    # depend on it, the bf16 copy only feeds the later h.
                    # During the pipeline fill DVE is idle - let it take the
                    # bf16 copies so psum recycles sooner.
                    nc.scalar.copy(c8t[:, pr, :, :], cp)
                    nc.scalar.copy(ct[:, 2 * pr:2 * pr + 2, :], cp)
                c_sb[i] = ct
                c8_sb[i] = c8t

            def emit_rest(i):
                ott = opool.tile([P, NOT, CH], BF16, tag="outt")
                last = i == NCH - 1
                halves = 2 if last else 1
                HW_ = CH // halves

                def emit_g(ot):
                    gp = gps_pool.tile([P, CH], F32, tag="gps")
                    for pr in range(NPAIR):
                        nc.tensor.matmul(
                            gp,
                            lhsT=gw8_sb[:, ot, pr, :, :],
                            rhs=c8_sb[i][:, pr, :, :],
                            start=(pr == 0), stop=(pr == NPAIR - 1),
                            perf_mode=PM.DoubleRow,
                        )
                    at = apool.tile([P, CH], BF16, tag="at")
                    zt = zpool.tile([P, CH], BF16, tag="zt")
                    for hf in range(halves):
                        sl = slice(hf * HW_, (hf + 1) * HW_)
                        nc.scalar.activation(at[:, sl], gp[:, sl], AF.Sigmoid,
                                             bias=(gbn0[:, :] if ot == 0
                                                   else 0.0),
                                             scale=-1.0)
                        # at the drain DVE's inline z shortens the relay;
                        # mid-stream Pool takes z to keep DVE de-saturated
                        zeng = nc.vector if last else nc.gpsimd
                        zeng.tensor_scalar(zt[:, sl], at[:, sl],
                                           -1.0, 1.0, OP.mult, OP.add)
                    return at, zt

                def emit_h(ot, at, zt):
                    if last and ot >= 2:
                        # g psum is done at the drain (all-g-first): the last
                        # two h matmuls take gps tiles so they never wait on
                        # hps recycling behind the DVE v-mult backlog
                        hp = gps_pool.tile([P, CH], F32, tag="gps",
                                           name=f"hx{ot}")
                    else:
                        hp = hps_pool.tile([P, CH], F32, tag="hps")
                    for dt in range(NDT):
                        nc.tensor.matmul(
                            hp,
                            lhsT=hwTr_sb[:, ot * 512 + dt * P:
                                         ot * 512 + dt * P + P],
                            rhs=c_sb[i][:, dt, :],
                            start=(dt == 0), stop=(dt == NDT - 1),
                        )
                    vt = vpool.tile([P, CH], BF16, tag="vt")
                    for hf in range(halves):
                        sl = slice(hf * HW_, (hf + 1) * HW_)
                        nc.vector.tensor_tensor(vt[:, sl], zt[:, sl],
                                                hp[:, sl], OP.mult)
                        if hf == 0:
                            init = (0.0 if i == 0
                                    else prev_out[0][:, ot, CH - 1:CH])
                        else:
                            init = ott[:, ot, hf * HW_ - 1:hf * HW_]
                        nc.vector.tensor_tensor_scan(
                            ott[:, ot, sl], at[:, sl], vt[:, sl], init,
                            OP.mult, OP.add)
                        if last:
                            # Sync is idle at the drain; keep Scalar's queue
                            # free for the sigmoid/z chain
                            nc.sync.dma_start(
                                out=outr3[:, ot, i * CH + hf * HW_:
                                          i * CH + (hf + 1) * HW_],
                                in_=ott[:, ot, sl])

                if i >= NCH - 2:
                    # all g matmuls first: every o-tile's sigmoid/z finishes
                    # while the h matmuls still stream, shortening the drain
                    az = [emit_g(ot) for ot in range(NOT)]
                    for ot in range(NOT):
                        emit_h(ot, *az[ot])
                else:
                    for ot in range(NOT):
                        at, zt = emit_g(ot)
                        emit_h(ot, at, zt)
                if not last:
                    # Sync queue: Scalar would head-of-line-block the conv
                    # psum->sbuf copies behind this store's scan dependency
                    nc.sync.dma_start(
                        out=outr3[:, :, i * CH:(i + 1) * CH], in_=ott)
                prev_out[0] = ott

            def emit_rest_last():
                # last chunk: column-split the g/h MATMULS too, so half-A's
                # v/scan chain runs on DVE while PE still streams half-B -
                # the post-PE DVE tail shrinks to one half-chunk's relay
                i = NCH - 1
                ott = opool.tile([P, NOT, CH], BF16, tag="outt")
                HW2 = CH // 2
                at_t, zt_t = {}, {}
                for ot in range(NOT):
                    at_t[ot] = apool.tile([P, CH], BF16, tag="at",
                                          name=f"at7_{ot}")
                    zt_t[ot] = zpool.tile([P, CH], BF16, tag="zt",
                                          name=f"zt7_{ot}")
                for hf in range(2):
                    sl = slice(hf * HW2, (hf + 1) * HW2)
                    for ot in range(NOT):
                        gp = gps_pool.tile([P, HW2], F32, tag="gps",
                                           name=f"g7_{ot}_{hf}")
                        for pr in range(NPAIR):
                            nc.tensor.matmul(
                                gp, lhsT=gw8_sb[:, ot, pr, :, :],
                                rhs=c8_sb[i][:, pr, :, sl],
                                start=(pr == 0), stop=(pr == NPAIR - 1),
                                perf_mode=PM.DoubleRow)
                        nc.scalar.activation(at_t[ot][:, sl], gp, AF.Sigmoid,
                                             bias=(gbn0[:, :] if ot == 0
                                                   else 0.0),
                                             scale=-1.0)
                        nc.vector.tensor_scalar(zt_t[ot][:, sl],
                                                at_t[ot][:, sl],
                                                -1.0, 1.0, OP.mult, OP.add)
                    for ot in range(NOT):
                        if ot >= 2:
                            hp = gps_pool.tile([P, HW2], F32, tag="gps",
                                               name=f"hx7_{ot}_{hf}")
                        else:
                            hp = hps_pool.tile([P, HW2], F32, tag="hps",
                                               name=f"h7_{ot}_{hf}")
                        for dt in range(NDT):
                            nc.tensor.matmul(
                                hp,
                                lhsT=hwTr_sb[:, ot * 512 + dt * P:
                                             ot * 512 + dt * P + P],
                                rhs=c_sb[i][:, dt, sl],
                                start=(dt == 0), stop=(dt == NDT - 1))
                        vt = vpool.tile([P, HW2], BF16, tag="vt",
                                        name=f"v7_{ot}_{hf}")
                        nc.vector.tensor_tensor(vt, zt_t[ot][:, sl], hp,
                                                OP.mult)
                        init = (prev_out[0][:, ot, CH - 1:CH] if hf == 0
                                else ott[:, ot, HW2 - 1:HW2])
                        nc.vector.tensor_tensor_scan(
                            ott[:, ot, sl], at_t[ot][:, sl], vt, init,
                            OP.mult, OP.add)
                        nc.sync.dma_start(
                            out=outr3[:, ot, i * CH + hf * HW2:
                                      i * CH + (hf + 1) * HW2],
                            in_=ott[:, ot, sl])
                prev_out[0] = ott

            # conv matmuls run two chunks ahead of rest; the psum->sbuf
            # copies are emitted separately so chunk i's sigmoids only sit
            # behind chunk i+1's copies in the Act queue (not i+1 and i+2)
            emit_conv_mm(0)
            emit_conv_copies(0)
            emit_conv_mm(1)
            for i in range(NCH - 2):
                emit_rest(i)
                emit_conv_copies(i + 1)
                emit_conv_mm(i + 2)
            emit_rest(NCH - 2)
            emit_conv_copies(NCH - 1)
            emit_rest(NCH - 1)

    nc.finalize()
    return nc


_PROGRAM = None


def _get_program():
    global _PROGRAM
    if _PROGRAM is None:
        _PROGRAM = build_program()
    return _PROGRAM


def prepare_in_maps(x, conv_w, h_w, g_w):
    x = np.ascontiguousarray(np.asarray(x), dtype=np.float32)
    conv_w = np.asarray(conv_w, dtype=np.float32)
    h_w = np.asarray(h_w, dtype=np.float32)
    g_w = np.asarray(g_w, dtype=np.float32)

    # hwTr[p, ot*512 + dt*128 + m] = h_w[ot*128+m, dt*128+p]  (bf16)
    hw = h_w[:, :, 0]                                             # [O, D]
    hwTr = np.ascontiguousarray(
        hw.reshape(NOT, P, NDT, P).transpose(3, 0, 2, 1).reshape(P, -1)
        .astype(ml_dtypes.bfloat16))

    # gw8[p, ot, pair, j, m] = gwp[ot*128+m, (2*pair+j)*128+p]  (fp8 e4m3)
    gwp = np.zeros((O, D), np.float32)
    gwp[2:, :] = g_w[:, :, 0]
    gw8 = np.ascontiguousarray(
        gwp.reshape(NOT, P, NPAIR, 2, P).transpose(4, 0, 2, 3, 1)
        .reshape(P, -1).astype(ml_dtypes.float8_e4m3))

    # cwd[p, (dt*5+k)*128 + q] = (q == p) * conv_w[dt*128+p, 0, k], with the
    # odd d-tiles' diagonal rolled by 64 partitions (matmul tile packing).
    cwd = np.zeros((P, NDT * NTAPS, P), np.float32)
    q = np.arange(P)
    for dt in range(NDT):
        pq = (q + 64) % P if dt % 2 else q
        for k in range(NTAPS):
            cwd[pq, dt * NTAPS + k, q] = conv_w[dt * P + q, 0, k]
    cwd = np.ascontiguousarray(cwd.reshape(P, -1).astype(ml_dtypes.bfloat16))

    # xrh[p, i, dt, j] = xpad[dt*128+p, i*512 + j] with 2-col zero halos;
    # odd d-tiles partition-rolled by 64.
    in_maps = []
    for b in range(B):
        xpad = np.pad(x[b], ((0, 0), (2, 2))).astype(ml_dtypes.bfloat16)
        xpad = xpad.reshape(NDT, P, L + 4)
        xroll = np.empty_like(xpad)
        for dt in range(NDT):
            xroll[dt] = np.roll(xpad[dt], 64, axis=0) if dt % 2 else xpad[dt]
        xb = np.empty((P, NCH, NDT, XW), dtype=ml_dtypes.bfloat16)
        for i in range(NCH):
            xb[:, i, :, :] = xroll[:, :, i * CH:i * CH + XW].transpose(1, 0, 2)
        in_maps.append({"xrh": np.ascontiguousarray(xb.reshape(P, -1)),
                        "cwd": cwd, "hwTr": hwTr, "gw8": gw8})
    return in_maps


def kernel(x, conv_w, h_w, g_w):
    in_maps = prepare_in_maps(x, conv_w, h_w, g_w)
    nc = _get_program()
    res = run_bass_kernel_spmd(nc, in_maps, list(range(N_CORES))).results
    return np.stack(
        [res[b]["outr"].reshape(P, NOT, L).transpose(1, 0, 2).reshape(O, L)
         .astype(np.float32)
         for b in range(B)], axis=0)
